# revision 1
# baseline (speedup 1.0000x reference)
"""Trainium2 Bass kernel for nn_Attention_80917183857290.

Multi-head causal attention (B=2, S=2048, D=1024, H=32, HD=32) with
SSMax-style per-query log-length score scaling, run SPMD on 8 NeuronCores.

Sharding: core c -> batch b = c // 4, head-group g2 = c % 4 (8 heads each).
Per core:
  - projections q,k (transposed layout [head_dim rows, seq]) and v
  - scores^T[k, q] per head via row-tiled K=32 matmuls (4 heads concurrent)
  - probs = exp(scores * sll * ss / sqrt(hd)) * exp(mask)  (no max-subtraction;
    fp32 exp range is sufficient for these magnitudes)
  - out^T[hd, q] and softmax denominators in one pass: V is ones-augmented
    ([v_h | 1], M=33) and PV runs as 64-stride col-tiled matmuls; a pair of
    host-built permutation matmuls then compacts the 4 heads + replicated
    denominators into one PSUM bank each
  - att^T = out^T * recip_approx(denom); partial = att^T.T @ wo_shard via PE
  - host sums the 4 partial outputs per batch. All dense matmuls use f32r
    (TF32-like single-pass fp32, 4x the fp32 PE rate; inputs pre-rounded).

The mask enters as exp(mask) tiles classified host-side per [128, 512] tile of
mask^T: all-zero tiles are skipped entirely (no scores/exp/PV), all-one tiles
skip the multiply, others are multiplied in bf16. exp(s+m) == exp(s)*exp(m).
"""

import math

import numpy as np
import ml_dtypes

B, S, D, H = 2, 2048, 1024, 32
HD = D // H  # 32
P = 128
QC = 512  # q-chunk (PSUM bank free size, fp32)
NQC = S // QC  # 4
NKT = S // P  # 16
NCORES = 8

_GRAPH_CACHE: dict = {}


def _build_graph(plans, nt, reps=1):
    """Build the per-core Bass graph.

    plans: tuple over qc (4) of tuple of (kt, mask_idx) entries; mask_idx -1
      means no mask multiply; >=0 indexes the packed emask tile array.
    nt: number of packed [128, 512] bf16 exp-mask tiles (>= 1).
    """
    import concourse.mybir as mybir
    from concourse import bacc
    from concourse.tile import TileContext

    f32 = mybir.dt.float32
    f32r = mybir.dt.float32r
    bf16 = mybir.dt.bfloat16
    EXP = mybir.ActivationFunctionType.Exp
    MULT = mybir.AluOpType.mult

    nc = bacc.Bacc()

    xT = nc.declare_dram_parameter("xT", [D, S], f32r, isOutput=False)
    aq = nc.declare_dram_parameter("aq", [D, 256], f32r, isOutput=False)
    ak = nc.declare_dram_parameter("ak", [D, 256], f32r, isOutput=False)
    av = nc.declare_dram_parameter("av", [D, 256], f32r, isOutput=False)
    wor = nc.declare_dram_parameter("wor", [256, D], f32r, isOutput=False)
    sllb = nc.declare_dram_parameter("sllb", [P, S], f32, isOutput=False)
    emask = nc.declare_dram_parameter("emask", [nt, P, QC], bf16, isOutput=False)
    sel = nc.declare_dram_parameter("sel", [4, P, P], f32r, isOutput=False)
    out = nc.declare_dram_parameter("out", [S, D], bf16, isOutput=True)

    with TileContext(nc) as tc:
        with (
            tc.tile_pool(name="consts", bufs=1) as consts,
            tc.tile_pool(name="ps", bufs=4, space="PSUM") as ps_pool,
            tc.tile_pool(name="probs", bufs=4) as probs_pool,
            tc.tile_pool(name="emt", bufs=4) as emt_pool,
            tc.tile_pool(name="oout", bufs=3) as oout_pool,
            tc.tile_pool(name="recip", bufs=3) as recip_pool,
        ):
          for _rep in range(reps):
            # ---- constant loads (chunked so consumers start early) ----
            aq_sb = consts.tile([P, 8, 256], f32r, tag="aq", name="aq")
            nc.sync.dma_start(out=aq_sb[:], in_=aq.rearrange("(ko ki) f -> ki ko f", ki=P))
            xk = []
            xT_r = xT.rearrange("(ko ki) f -> ki ko f", ki=P)
            for k in range(8):
                t = consts.tile([P, S], f32r, tag=f"xk{k}", name=f"xk{k}")
                nc.sync.dma_start(out=t[:], in_=xT_r[:, k])
                xk.append(t)
            ak_sb = consts.tile([P, 8, 256], f32r, tag="ak", name="ak")
            nc.sync.dma_start(out=ak_sb[:], in_=ak.rearrange("(ko ki) f -> ki ko f", ki=P))
            av_sb = consts.tile([P, 8, 256], f32r, tag="av", name="av")
            nc.sync.dma_start(out=av_sb[:], in_=av.rearrange("(ko ki) f -> ki ko f", ki=P))
            sll_sb = consts.tile([P, S], f32, tag="sll", name="sll")
            nc.sync.dma_start(out=sll_sb[:], in_=sllb[:])
            sel_sb = consts.tile([P, 4, P], f32r, tag="sel", name="sel")
            nc.sync.dma_start(out=sel_sb[:], in_=sel.rearrange("t k m -> k t m"))
            wor_sb = consts.tile([P, 2, D], f32r, tag="wor", name="wor")
            nc.sync.dma_start(out=wor_sb[:], in_=wor.rearrange("(ko ki) f -> ki ko f", ki=P))
            em_res = None
            if nt <= 16:
                em_res = consts.tile([P, nt, QC], bf16, tag="emres", name="emres")
                nc.sync.dma_start(out=em_res[:], in_=emask.rearrange("t p f -> p t f"))

            # per-(group, half) q^T/k^T tiles, per-quarter v tiles,
            # per-(group, qc) attention outputs: chunk granularity lets the
            # scheduler overlap projections with attention on earlier chunks.
            qTh = [[consts.tile([P, 1024], f32r, tag=f"qT{g}{h}", name=f"qT{g}{h}") for h in range(2)] for g in range(2)]
            kTh = [[consts.tile([P, 1024], f32r, tag=f"kT{g}{h}", name=f"kT{g}{h}") for h in range(2)] for g in range(2)]
            vq = [consts.tile([P, 4, 8, 33], bf16, tag=f"vq{q}", name=f"vq{q}") for q in range(4)]
            attQ = [[consts.tile([P, QC], f32r, tag=f"att{g}{q}", name=f"att{g}{q}") for q in range(NQC)] for g in range(2)]

            def proj_qk_unit(w, g, half, c2):
                # one 512-wide chunk of the q or k projection for (g, half)
                lhs_sb = aq_sb if w == "q" else ak_sb
                base = 1024 * half + 512 * c2
                ps = ps_pool.tile([P, 1024], f32, tag="ps", name="ps")
                for k in range(8):
                    nc.tensor.matmul(
                        ps[:, :512],
                        lhsT=lhs_sb[:, k, 128 * g : 128 * g + 128],
                        rhs=xk[k][:, base : base + 512],
                        start=(k == 0),
                        stop=(k == 7),
                    )
                dst = (qTh if w == "q" else kTh)[g][half][:, 512 * c2 : 512 * c2 + 512]
                if w == "q":
                    nc.vector.tensor_tensor(dst, ps[:, :512], sll_sb[:, base : base + 512], MULT)
                else:
                    nc.vector.tensor_copy(dst, ps[:, :512])

            def qk_half_units(half):
                return [
                    (lambda w=w, g=g, c2=c2: proj_qk_unit(w, g, half, c2))
                    for g in range(2)
                    for w in ("q", "k")
                    for c2 in range(2)
                ]

            def proj_v_unit(qq, sti):
                if sti == 0:
                    nc.vector.memset(vq[qq][:], 1.0)
                st = 4 * qq + sti
                psv = ps_pool.tile([P, 1024], f32, tag="ps", name="ps")
                for k in range(8):
                    nc.tensor.matmul(
                        psv[:, :256],
                        lhsT=xk[k][:, 128 * st : 128 * st + 128],
                        rhs=av_sb[:, k, :],
                        start=(k == 0),
                        stop=(k == 7),
                    )
                nc.vector.tensor_copy(
                    vq[qq][:, sti, :, 0:32],
                    psv[:, :256].rearrange("p (h c) -> p h c", h=8),
                )

            def v_quarter_units(qq):
                return [(lambda qq=qq, sti=sti: proj_v_unit(qq, sti)) for sti in range(4)]

            def attention_qc(qc, fillers):
                entries = plans[qc]
                qh, qcol = qc // 2, 512 * (qc % 2)
                for g in range(2):
                    if not entries:
                        nc.vector.memset(attQ[g][qc][:], 0.0)
                        continue
                    pvdn = ps_pool.tile([P, 1024], f32, tag="ps", name="ps")
                    # zero accumulator banks; matmuls use start=False so the
                    # per-element has_written state (set -> add onto 0,
                    # unset -> overwrite) is correct in any order.
                    nc.vector.memset(pvdn[:], 0.0)
                    nent = len(entries)
                    for ei, (kt, mi, zq) in enumerate(entries):
                        if fillers and ei % 2 == 1:
                            fillers.popleft()()
                        kh, kcol = kt // 8, 128 * (kt % 8)
                        w = QC - zq  # leading zq q-columns are fully masked
                        psa = ps_pool.tile([P, 1024], f32, tag="ps", name="ps")
                        psb = ps_pool.tile([P, 1024], f32, tag="ps", name="ps")
                        for j in range(4):
                            dst = (psa if j < 2 else psb)[:, 512 * (j % 2) + zq : 512 * (j % 2) + 512]
                            nc.tensor.matmul(
                                dst,
                                lhsT=kTh[g][kh][32 * j : 32 * j + 32, kcol : kcol + 128],
                                rhs=qTh[g][qh][32 * j : 32 * j + 32, qcol + zq : qcol + 512],
                                start=True,
                                stop=True,
                                tile_position=(32 * j, 0),
                            )
                        pr = probs_pool.tile([P, 2048], bf16, tag="pr", name="pr")
                        if zq == 0:
                            nc.scalar.activation(pr[:, :1024], psa[:], EXP)
                            nc.scalar.activation(pr[:, 1024:], psb[:], EXP)
                        else:
                            for j in range(4):
                                ps_ = (psa if j < 2 else psb)[:, 512 * (j % 2) + zq : 512 * (j % 2) + 512]
                                nc.scalar.activation(pr[:, 512 * j + zq : 512 * j + 512], ps_, EXP)
                        if mi >= 0:
                            if em_res is not None:
                                emt = em_res[:, mi, :]
                            else:
                                emtt = emt_pool.tile([P, QC], bf16, tag="emt", name="emt")
                                nc.sync.dma_start(out=emtt[:], in_=emask[mi])
                                emt = emtt[:]
                            pr3 = pr[:].rearrange("p (h f) -> p h f", h=4)[:, :, zq:]
                            nc.vector.tensor_tensor(
                                pr3, pr3, emt[:, None, zq:].to_broadcast((P, 4, w)), MULT
                            )
                        last = ei == nent - 1
                        for j in range(4):
                            bank = pvdn[:, :512] if j < 2 else pvdn[:, 512:]
                            idx = j % 2
                            nc.tensor.matmul(
                                bank[64 * idx : 64 * idx + 33, zq:],
                                lhsT=vq[kt // 4][:, kt % 4, 4 * g + j, :],
                                rhs=pr[:, 512 * j + zq : 512 * j + 512],
                                start=False,
                                stop=(last and j == 3),
                                tile_position=(0, 64 * idx),
                                skip_group_check=True,
                            )
                    sab = recip_pool.tile([P, 1024], f32r, tag="sab", name="sab")
                    nc.vector.tensor_copy(sab[:], pvdn[:])
                    # compact back into pvdn's own banks (start=True re-clears)
                    pvC = pvdn[:, :512]
                    dnC = pvdn[:, 512:]
                    nc.tensor.matmul(pvC, lhsT=sel_sb[:, 0], rhs=sab[:, :512], start=True, stop=False)
                    nc.tensor.matmul(pvC, lhsT=sel_sb[:, 1], rhs=sab[:, 512:], start=False, stop=True)
                    nc.tensor.matmul(dnC, lhsT=sel_sb[:, 2], rhs=sab[:, :512], start=True, stop=False)
                    nc.tensor.matmul(dnC, lhsT=sel_sb[:, 3], rhs=sab[:, 512:], start=False, stop=True)
                    rc = recip_pool.tile([P, QC], f32, tag="rc", name="rc")
                    nc.vector.reciprocal_approx_fast(out=rc[:], in_=dnC)
                    nc.vector.tensor_tensor(attQ[g][qc][:], pvC, rc[:], MULT)

            def wo_qc(qc):
                for sti in range(4):
                    st = 4 * qc + sti
                    ob = oout_pool.tile([P, D], bf16, tag="ob", name="ob")
                    wops = ps_pool.tile([P, 1024], f32, tag="ps", name="ps")
                    for n in range(2):
                        dst = wops[:, 512 * n : 512 * n + 512]
                        for kk in range(2):
                            nc.tensor.matmul(
                                dst,
                                lhsT=attQ[kk][qc][:, 128 * sti : 128 * sti + 128],
                                rhs=wor_sb[:, kk, 512 * n : 512 * n + 512],
                                start=(kk == 0),
                                stop=(kk == 1),
                            )
                    if sti % 2 == 0:
                        nc.vector.tensor_copy(ob[:], wops[:])
                    else:
                        nc.scalar.copy(ob[:], wops[:])
                    nc.sync.dma_start(out=out[128 * st : 128 * st + 128, :], in_=ob[:])

            # emission order interleaves projections with attention chunks so
            # ACT starts exping as soon as the first chunk's q/k/v exist.
            # Resource-tracked emission: each q/k half and v quarter is
            # emitted as small units, woven between attention kt-iterations
            # of the PREVIOUS chunk where possible so ACT never starves; any
            # units still pending when a chunk actually needs them are
            # drained first (plan-dependent, so non-causal masks stay
            # correct).
            from collections import deque

            done: set = set()
            queued: dict = {}
            fillers: deque = deque()

            def resource_units(r):
                kind, idx = r
                return qk_half_units(idx) if kind == "qk" else v_quarter_units(idx)

            def needs(qc):
                res = [("qk", qc // 2)]
                res += [("qk", kt // 8) for kt, _, _ in plans[qc]]
                res += [("v", kt // 4) for kt, _, _ in plans[qc]]
                seen = []
                for r in res:
                    if r not in seen:
                        seen.append(r)
                return seen

            def drain(r):
                if r in done:
                    return
                for u in queued.pop(r, None) or resource_units(r):
                    u()
                done.add(r)

            def queue(r):
                if r in done or r in queued:
                    return
                units = resource_units(r)
                queued[r] = []
                fillers.extend(units)
                done.add(r)  # fully queued counts as emitted-by-the-time-we-drain

            prev = None
            for qc in range(NQC):
                for r in needs(qc):
                    # required now: flush any queued-but-unemitted fillers
                    while fillers:
                        fillers.popleft()()
                    drain(r)
                if qc + 1 < NQC:
                    for r in needs(qc + 1):
                        if r not in done:
                            queue(r)
                attention_qc(qc, fillers)
                while fillers:
                    fillers.popleft()()
                if prev is not None:
                    wo_qc(prev)
                prev = qc
            wo_qc(prev)

    if not nc.is_finalized():
        nc.finalize()
    return nc


def _round_f32r(a):
    """Round fp32 array to the PE's f32r format (mantissa truncated to 11
    bits, round-to-nearest-even at bit 12) so f32r-declared DMA inputs match
    what an on-device cast would produce."""
    u = np.ascontiguousarray(a, dtype=np.float32).view(np.uint32)
    u2 = (u + np.uint32(0x7FF) + ((u >> np.uint32(12)) & np.uint32(1))) & np.uint32(0xFFFFF000)
    return u2.view(np.float32)


def _plan_from_mask(mask):
    """Classify [128, 512] tiles of exp(mask)^T; returns (plans, packed_tiles)."""
    em = np.exp(mask.astype(np.float32))  # [q, k]
    emT = np.ascontiguousarray(em.T)  # [k, q]
    plans = []
    tiles = []
    tile_keys = {}
    for qc in range(NQC):
        ent = []
        for kt in range(NKT):
            t = emT[P * kt : P * (kt + 1), QC * qc : QC * (qc + 1)]
            if not t.any():
                continue  # fully masked out: skip tile entirely
            if (t == 1.0).all():
                ent.append((kt, -1, 0))
                continue
            # leading fully-masked q-columns can be skipped (even count for
            # bf16 slice alignment)
            nz = np.flatnonzero(t.any(axis=0))
            zq = (int(nz[0]) // 2 * 2) if len(nz) else 0
            key = t.tobytes()
            mi = tile_keys.get(key)
            if mi is None:
                mi = len(tiles)
                tile_keys[key] = mi
                tiles.append(t.astype(ml_dtypes.bfloat16))
            ent.append((kt, mi, zq))
        if ent and min(z for _, _, z in ent) > 0:
            # every tile skips some leading columns -> those pv/dn columns
            # would never be written; disable skipping for this chunk
            ent = [(kt, mi, 0) for kt, mi, _ in ent]
        plans.append(tuple(ent))
    if tiles:
        packed = np.ascontiguousarray(np.stack(tiles))
    else:
        packed = np.zeros((1, P, QC), dtype=ml_dtypes.bfloat16)
    return tuple(plans), packed


def _sel_mats():
    s = np.zeros((4, P, P), dtype=np.float32)
    for m in range(32):
        s[0, m, m] = 1.0            # pvC rows 0-31   <- bankA rows 0-31
        s[0, m + 64, m + 32] = 1.0  # pvC rows 32-63  <- bankA rows 64-95
        s[1, m, m + 64] = 1.0       # pvC rows 64-95  <- bankB rows 0-31
        s[1, m + 64, m + 96] = 1.0  # pvC rows 96-127 <- bankB rows 64-95
    s[2, 32, 0:32] = 1.0            # dnC rows 0-31   <- bankA row 32
    s[2, 96, 32:64] = 1.0           # dnC rows 32-63  <- bankA row 96
    s[3, 32, 64:96] = 1.0           # dnC rows 64-95  <- bankB row 32
    s[3, 96, 96:128] = 1.0          # dnC rows 96-127 <- bankB row 96
    return s


def kernel(x, mask, section_log_len, wq, wk, wv, wo, seq_scale):
    from concourse.bass_utils import run_bass_kernel_spmd

    x = np.asarray(x, dtype=np.float32)
    assert x.shape == (B, S, D), x.shape
    mask2 = np.asarray(mask, dtype=np.float32).reshape(S, S)
    sll = np.asarray(section_log_len, dtype=np.float32).reshape(S)
    ss = np.asarray(seq_scale, dtype=np.float32).reshape(H)
    wq = np.asarray(wq, dtype=np.float32)
    wk = np.asarray(wk, dtype=np.float32)
    wv = np.asarray(wv, dtype=np.float32)
    wo = np.asarray(wo, dtype=np.float32)

    plans, tiles = _plan_from_mask(mask2)
    key = (plans, tiles.shape[0])
    nc = _GRAPH_CACHE.get(key)
    if nc is None:
        nc = _build_graph(plans, tiles.shape[0])
        _GRAPH_CACHE[key] = nc

    sllB = np.ascontiguousarray(
        np.broadcast_to(sll[None, :], (P, S)), dtype=np.float32
    )
    xT = [_round_f32r(x[b].T) for b in range(B)]
    selm = _sel_mats()

    in_maps = []
    for c in range(NCORES):
        b, g2 = divmod(c, 4)
        rows = slice(256 * g2, 256 * (g2 + 1))
        ssr = np.repeat(ss[8 * g2 : 8 * g2 + 8], HD) / math.sqrt(HD)
        in_maps.append(
            {
                "xT": xT[b],
                "aq": _round_f32r((wq[rows, :] * ssr[:, None]).T),
                "ak": _round_f32r(wk[rows, :].T),
                "av": _round_f32r(wv[rows, :].T),
                "wor": _round_f32r(wo[:, rows].T),
                "sllb": sllB,
                "emask": tiles,
                "sel": selm,
            }
        )

    res = run_bass_kernel_spmd(nc, in_maps, core_ids=list(range(NCORES))).results
    out = np.zeros((B, S, D), dtype=np.float32)
    for c in range(NCORES):
        out[c // 4] += np.asarray(res[c]["out"], dtype=np.float32)
    return out



# revision 8
# speedup vs baseline: 1.0469x; 1.0469x over previous
"""Trainium2 Bass kernel for nn_Attention_80917183857290.

Multi-head causal attention (B=2, S=2048, D=1024, H=32, HD=32) with
SSMax-style per-query log-length score scaling, run SPMD on 8 NeuronCores.

Sharding: core c -> batch b = c // 4, head-group g2 = c % 4 (8 heads each).
Per core:
  - projections q,k (transposed layout [head_dim rows, seq]) and v
  - scores^T[k, q] per head via row-tiled K=32 matmuls (4 heads concurrent)
  - probs = exp(scores * sll * ss / sqrt(hd)), masked multiplicatively
  - PV with probs as the stationary operand: out [128q, 32hd] per
    (head, kt) accumulated in PSUM across kt -- the narrow free dim (32)
    makes PV ~4x cheaper on the PE than the scoresT-layout PV.
    Denominators via per-head [128q, 1] matmuls against a ones column.
  - att = pv * recip(dn); att^T via PE transpose; out = att^T.T @ wo_shard.
  - host sums the 4 partial outputs per batch. Dense matmuls use f32r
    (TF32-like single-pass fp32, inputs pre-rounded).

The causal mask reduces to a single shared [128,128] tril tile applied only
on diagonal-crossing blocks; fully-masked [128 k, 128 q] blocks are skipped
in scores/exp/PV entirely. Non-causal masks fall back to per-tile bf16
exp(mask) multiplies (correct for any mask).
"""

import math

import numpy as np
import ml_dtypes

B, S, D, H = 2, 2048, 1024, 32
HD = D // H  # 32
P = 128
QC = 512  # q-chunk (PSUM bank free size, fp32)
NQC = S // QC  # 4
NKT = S // P  # 16
NCORES = 8

_GRAPH_CACHE: dict = {}


def _build_graph(plans, nt, reps=1):
    """Build the per-core Bass graph.

    plans: tuple over qc (4) of tuple of (kt, mi, zq, tri) entries; mi -1
      means no general-mask multiply; >=0 indexes the packed emask tiles;
      tri means multiply the shared [128,128] tril tile on block zq//128.
      zq is a multiple of 128: leading zq q-columns are fully masked.
    nt: number of packed [128, 512] bf16 exp-mask tiles (>= 1).
    """
    import concourse.mybir as mybir
    from concourse import bacc
    from concourse.tile import TileContext

    f32 = mybir.dt.float32
    f32r = mybir.dt.float32r
    bf16 = mybir.dt.bfloat16
    EXP = mybir.ActivationFunctionType.Exp
    MULT = mybir.AluOpType.mult

    nc = bacc.Bacc()

    xT = nc.declare_dram_parameter("xT", [D, S], f32r, isOutput=False)
    aq = nc.declare_dram_parameter("aq", [D, 256], f32r, isOutput=False)
    ak = nc.declare_dram_parameter("ak", [D, 256], f32r, isOutput=False)
    av = nc.declare_dram_parameter("av", [D, 256], f32r, isOutput=False)
    wor = nc.declare_dram_parameter("wor", [256, D], f32r, isOutput=False)
    sllb = nc.declare_dram_parameter("sllb", [P, S], f32, isOutput=False)
    tril = nc.declare_dram_parameter("tril", [P, P], bf16, isOutput=False)
    ident = nc.declare_dram_parameter("ident", [P, P], bf16, isOutput=False)
    emask = nc.declare_dram_parameter("emask", [nt, P, QC], bf16, isOutput=False)
    out = nc.declare_dram_parameter("out", [S, D], bf16, isOutput=True)

    # first/last contributing entry index per (qc, qt) for pv/dn accumulation
    first_e = [[None] * 4 for _ in range(NQC)]
    last_e = [[None] * 4 for _ in range(NQC)]
    for qc in range(NQC):
        for ei, (kt, mi, zq, tri) in enumerate(plans[qc]):
            for qt in range(zq // P, 4):
                if first_e[qc][qt] is None:
                    first_e[qc][qt] = ei
                last_e[qc][qt] = ei

    with TileContext(nc) as tc:
        with (
            tc.tile_pool(name="consts", bufs=1) as consts,
            tc.tile_pool(name="sc", bufs=2, space="PSUM") as sc_pool,
            tc.tile_pool(name="pv", bufs=1, space="PSUM") as pv_pool,
            tc.tile_pool(name="ms", bufs=1, space="PSUM") as ms_pool,
            tc.tile_pool(name="ax", bufs=1, space="PSUM") as ax_pool,
            tc.tile_pool(name="probs", bufs=4) as probs_pool,
            tc.tile_pool(name="emt", bufs=4) as emt_pool,
            tc.tile_pool(name="att", bufs=4) as att_pool,
            tc.tile_pool(name="attT", bufs=2) as attT_pool,
            tc.tile_pool(name="oout", bufs=4) as oout_pool,
            tc.tile_pool(name="recip", bufs=2) as recip_pool,
        ):
          for _rep in range(reps):
            # ---- constant loads (chunked so consumers start early) ----
            aq_sb = consts.tile([P, 8, 256], f32r, tag="aq", name="aq")
            nc.sync.dma_start(out=aq_sb[:], in_=aq.rearrange("(ko ki) f -> ki ko f", ki=P))
            xk = []
            xT_r = xT.rearrange("(ko ki) f -> ki ko f", ki=P)
            for k in range(8):
                t = consts.tile([P, S], f32r, tag=f"xk{k}", name=f"xk{k}")
                nc.sync.dma_start(out=t[:], in_=xT_r[:, k])
                xk.append(t)
            ak_sb = consts.tile([P, 8, 256], f32r, tag="ak", name="ak")
            nc.sync.dma_start(out=ak_sb[:], in_=ak.rearrange("(ko ki) f -> ki ko f", ki=P))
            av_sb = consts.tile([P, 8, 256], f32r, tag="av", name="av")
            nc.sync.dma_start(out=av_sb[:], in_=av.rearrange("(ko ki) f -> ki ko f", ki=P))
            sll_sb = consts.tile([P, S], f32, tag="sll", name="sll")
            nc.sync.dma_start(out=sll_sb[:], in_=sllb[:])
            tril_sb = consts.tile([P, P], bf16, tag="tril", name="tril")
            nc.sync.dma_start(out=tril_sb[:], in_=tril[:])
            id_sb = consts.tile([P, P], bf16, tag="ident", name="ident")
            nc.sync.dma_start(out=id_sb[:], in_=ident[:])
            wor_sb = consts.tile([P, 2, D], f32r, tag="wor", name="wor")
            nc.sync.dma_start(out=wor_sb[:], in_=wor.rearrange("(ko ki) f -> ki ko f", ki=P))
            em_res = None
            if nt <= 16:
                em_res = consts.tile([P, nt, QC], bf16, tag="emres", name="emres")
                nc.sync.dma_start(out=em_res[:], in_=emask.rearrange("t p f -> p t f"))
            ones_sb = consts.tile([P, 1], bf16, tag="ones", name="ones")
            nc.vector.memset(ones_sb[:], 1.0)

            # per-(group, half) q^T/k^T tiles, per-quarter v tiles; chunk
            # granularity lets the scheduler overlap projections with
            # attention on earlier chunks.
            qTh = [[consts.tile([P, 1024], f32r, tag=f"qT{g}{h}", name=f"qT{g}{h}") for h in range(2)] for g in range(2)]
            kTh = [[consts.tile([P, 1024], f32r, tag=f"kT{g}{h}", name=f"kT{g}{h}") for h in range(2)] for g in range(2)]
            vq = [consts.tile([P, 4, 8, 32], bf16, tag=f"vq{q}", name=f"vq{q}") for q in range(4)]

            def proj_qk_unit(w, g, half, c2):
                # one 512-wide chunk of the q or k projection for (g, half)
                lhs_sb = aq_sb if w == "q" else ak_sb
                base = 1024 * half + 512 * c2
                ps = ax_pool.tile([P, 512], f32, tag="ax", name="ax")
                for k in range(8):
                    nc.tensor.matmul(
                        ps[:],
                        lhsT=lhs_sb[:, k, 128 * g : 128 * g + 128],
                        rhs=xk[k][:, base : base + 512],
                        start=(k == 0),
                        stop=(k == 7),
                    )
                dst = (qTh if w == "q" else kTh)[g][half][:, 512 * c2 : 512 * c2 + 512]
                if w == "q":
                    nc.vector.tensor_tensor(dst, ps[:], sll_sb[:, base : base + 512], MULT)
                else:
                    nc.vector.tensor_copy(dst, ps[:])

            def qk_half_units(half):
                return [
                    (lambda w=w, g=g, c2=c2: proj_qk_unit(w, g, half, c2))
                    for g in range(2)
                    for w in ("q", "k")
                    for c2 in range(2)
                ]

            def proj_v_unit(qq, sti):
                st = 4 * qq + sti
                psv = ax_pool.tile([P, 512], f32, tag="ax", name="ax")
                for k in range(8):
                    nc.tensor.matmul(
                        psv[:, :256],
                        lhsT=xk[k][:, 128 * st : 128 * st + 128],
                        rhs=av_sb[:, k, :],
                        start=(k == 0),
                        stop=(k == 7),
                    )
                nc.vector.tensor_copy(
                    vq[qq][:, sti, :, :],
                    psv[:, :256].rearrange("p (h c) -> p h c", h=8),
                )

            def v_quarter_units(qq):
                return [(lambda qq=qq, sti=sti: proj_v_unit(qq, sti)) for sti in range(4)]

            # per-qc state created in attention_qc, consumed by epilogue_*
            pvt = {}   # qc -> (pv01, pv23) tiles [P, 2, 8, 32] f32 PSUM
            mst = {}   # qc -> dn tile [P, 512] f32 PSUM (dn in [:, :32])
            attq = {}  # qc -> list of 4 att tiles [P, 8, 32] bf16 SBUF

            def attention_qc(qc, fillers):
                entries = plans[qc]
                qh, qcol = qc // 2, 512 * (qc % 2)
                pv01 = pv_pool.tile([P, 2, 8, 32], f32, tag="pv01", name="pv01")
                pv23 = pv_pool.tile([P, 2, 8, 32], f32, tag="pv23", name="pv23")
                ms = ms_pool.tile([P, 512], f32, tag="ms", name="ms")
                pvt[qc] = (pv01, pv23)
                mst[qc] = ms
                for g in range(2):
                    for ei, (kt, mi, zq, tri) in enumerate(entries):
                        if fillers and ei % 2 == 1:
                            fillers.popleft()()
                        kh, kcol = kt // 8, 128 * (kt % 8)
                        szq = min(zq, 256)  # keep matmul/exp width >= 256
                        psa = sc_pool.tile([P, 1024], f32, tag="sc", name="sc")
                        psb = sc_pool.tile([P, 1024], f32, tag="sc", name="sc")
                        for j in range(4):
                            dst = (psa if j < 2 else psb)[:, 512 * (j % 2) + szq : 512 * (j % 2) + 512]
                            nc.tensor.matmul(
                                dst,
                                lhsT=kTh[g][kh][32 * j : 32 * j + 32, kcol : kcol + 128],
                                rhs=qTh[g][qh][32 * j : 32 * j + 32, qcol + szq : qcol + 512],
                                start=True,
                                stop=True,
                                tile_position=(32 * j, 0),
                            )
                        pr = probs_pool.tile([P, 2048], bf16, tag="pr", name="pr")
                        if szq == 0:
                            nc.scalar.activation(pr[:, :1024], psa[:], EXP)
                            nc.scalar.activation(pr[:, 1024:], psb[:], EXP)
                        else:
                            for j in range(4):
                                ps_ = (psa if j < 2 else psb)[:, 512 * (j % 2) + szq : 512 * (j % 2) + 512]
                                nc.scalar.activation(pr[:, 512 * j + szq : 512 * j + 512], ps_, EXP)
                        pr4 = pr[:].rearrange("p (h f) -> p h f", h=4)
                        if tri:
                            blk = pr4[:, :, zq : zq + P]
                            nc.vector.tensor_tensor(
                                blk, blk, tril_sb[:, None, :].to_broadcast((P, 4, P)), MULT
                            )
                        elif mi >= 0:
                            if em_res is not None:
                                emt = em_res[:, mi, :]
                            else:
                                emtt = emt_pool.tile([P, QC], bf16, tag="emt", name="emt")
                                nc.sync.dma_start(out=emtt[:], in_=emask[mi])
                                emt = emtt[:]
                            w = QC - zq
                            pr3 = pr4[:, :, zq:]
                            nc.vector.tensor_tensor(
                                pr3, pr3, emt[:, None, zq:].to_broadcast((P, 4, w)), MULT
                            )
                        for j in range(4):
                            h = 4 * g + j
                            for qt in range(zq // P, 4):
                                lhsT = pr[:, 512 * j + 128 * qt : 512 * j + 128 * qt + 128]
                                pvd = (pv01 if qt < 2 else pv23)[:, qt % 2, h, :]
                                nc.tensor.matmul(
                                    pvd,
                                    lhsT=lhsT,
                                    rhs=vq[kt // 4][:, kt % 4, h, :],
                                    start=(ei == first_e[qc][qt]),
                                    stop=(ei == last_e[qc][qt]),
                                    skip_group_check=True,
                                )
                                nc.tensor.matmul(
                                    ms[:, 8 * qt + h : 8 * qt + h + 1],
                                    lhsT=lhsT,
                                    rhs=ones_sb[:],
                                    start=(ei == first_e[qc][qt]),
                                    stop=(ei == last_e[qc][qt]),
                                    skip_group_check=True,
                                )

            def epilogue_divide(qc):
                # recip of the 32 denominators, then per-qt normalize to SBUF
                if not plans[qc]:
                    ats = []
                    for qt in range(4):
                        at = att_pool.tile([P, 8, 32], bf16, tag="at", name="at")
                        nc.vector.memset(at[:], 0.0)
                        ats.append(at)
                    attq[qc] = ats
                    return
                pv01, pv23 = pvt[qc]
                ms = mst[qc]
                rc = recip_pool.tile([P, 32], f32, tag="rc", name="rc")
                nc.vector.reciprocal_approx_fast(out=rc[:], in_=ms[:, :32])
                ats = []
                for qt in range(4):
                    at = att_pool.tile([P, 8, 32], bf16, tag="at", name="at")
                    nc.vector.tensor_tensor(
                        at[:],
                        (pv01 if qt < 2 else pv23)[:, qt % 2],
                        rc[:, 8 * qt : 8 * qt + 8, None].to_broadcast((P, 8, 32)),
                        MULT,
                    )
                    ats.append(at)
                attq[qc] = ats

            def epilogue_wo(qc):
                # transpose att per qt (PE), copy to SBUF f32r, then the
                # output projection; emission staggers transposes one qt
                # ahead of the wo matmuls so the aT copy latency is hidden.
                psTs, aTs = [], []

                def tr(qt):
                    at2 = attq[qc][qt][:].rearrange("p h c -> p (h c)")
                    axt = ax_pool.tile([P, 512], f32, tag="ax", name="ax")
                    psT = axt[:].bitcast(bf16)[:, 0:256].rearrange("p (c q) -> p c q", c=2)
                    for c in range(2):
                        nc.tensor.matmul(
                            psT[:, c],
                            lhsT=at2[:, 128 * c : 128 * c + 128],
                            rhs=id_sb[:],
                            is_transpose=True,
                        )
                    aT = attT_pool.tile([P, 2, 128], f32r, tag="aT", name="aT")
                    nc.vector.tensor_copy(aT[:], psT)
                    aTs.append(aT)

                def wo_qt(qt):
                    aT = aTs[qt]
                    st = 4 * qc + qt
                    for n in range(2):
                        wops = ax_pool.tile([P, 512], f32, tag="ax", name="ax")
                        for kk in range(2):
                            nc.tensor.matmul(
                                wops[:],
                                lhsT=aT[:, kk],
                                rhs=wor_sb[:, kk, 512 * n : 512 * n + 512],
                                start=(kk == 0),
                                stop=(kk == 1),
                            )
                        ob = oout_pool.tile([P, 512], bf16, tag="ob", name="ob")
                        nc.vector.tensor_copy(ob[:], wops[:])
                        nc.sync.dma_start(
                            out=out[128 * st : 128 * st + 128, 512 * n : 512 * n + 512],
                            in_=ob[:],
                        )

                tr(0)
                tr(1)
                wo_qt(0)
                tr(2)
                wo_qt(1)
                tr(3)
                wo_qt(2)
                wo_qt(3)

            # emission order interleaves projections with attention chunks so
            # ACT starts exping as soon as the first chunk's q/k/v exist.
            # Resource-tracked emission: each q/k half and v quarter is
            # emitted as small units, woven between attention kt-iterations
            # of the PREVIOUS chunk where possible so ACT never starves; any
            # units still pending when a chunk actually needs them are
            # drained first (plan-dependent, so non-causal masks stay
            # correct).
            from collections import deque

            done: set = set()
            queued: dict = {}
            fillers: deque = deque()

            def resource_units(r):
                kind, idx = r
                return qk_half_units(idx) if kind == "qk" else v_quarter_units(idx)

            def needs(qc):
                res = [("qk", qc // 2)]
                res += [("qk", kt // 8) for kt, _, _, _ in plans[qc]]
                res += [("v", kt // 4) for kt, _, _, _ in plans[qc]]
                seen = []
                for r in res:
                    if r not in seen:
                        seen.append(r)
                return seen

            def drain(r):
                if r in done:
                    return
                for u in queued.pop(r, None) or resource_units(r):
                    u()
                done.add(r)

            def queue(r):
                if r in done or r in queued:
                    return
                units = resource_units(r)
                queued[r] = []
                fillers.extend(units)
                done.add(r)  # fully queued counts as emitted-by-the-time-we-drain

            prev = None
            for qc in range(NQC):
                for r in needs(qc):
                    # required now: flush any queued-but-unemitted fillers
                    while fillers:
                        fillers.popleft()()
                    drain(r)
                if qc + 1 < NQC:
                    for r in needs(qc + 1):
                        if r not in done:
                            queue(r)
                if prev is not None:
                    epilogue_divide(prev)
                attention_qc(qc, fillers)
                while fillers:
                    fillers.popleft()()
                if prev is not None:
                    epilogue_wo(prev)
                prev = qc
            epilogue_divide(prev)
            epilogue_wo(prev)

    if not nc.is_finalized():
        nc.finalize()
    return nc


def _round_f32r(a):
    """Round fp32 array to the PE's f32r format (mantissa truncated to 11
    bits, round-to-nearest-even at bit 12) so f32r-declared DMA inputs match
    what an on-device cast would produce."""
    u = np.ascontiguousarray(a, dtype=np.float32).view(np.uint32)
    u2 = (u + np.uint32(0x7FF) + ((u >> np.uint32(12)) & np.uint32(1))) & np.uint32(0xFFFFF000)
    return u2.view(np.float32)


def _plan_from_mask(mask):
    """Classify [128, 512] tiles of exp(mask)^T; returns (plans, packed_tiles).

    Entries are (kt, mi, zq, tri): zq (multiple of 128) leading fully-masked
    q-columns; tri=True means the tile is [zeros | tril(128) | ones] so only
    the shared tril block needs multiplying; mi >= 0 indexes a packed general
    bf16 exp(mask) tile.
    """
    em = np.exp(mask.astype(np.float32))  # [q, k]
    emT = np.ascontiguousarray(em.T)  # [k, q]
    tril_blk = np.tril(np.ones((P, P), dtype=np.float32))
    plans = []
    tiles = []
    tile_keys = {}
    for qc in range(NQC):
        ent = []
        covered = [False] * 4
        for kt in range(NKT):
            t = emT[P * kt : P * (kt + 1), QC * qc : QC * (qc + 1)]
            if not t.any():
                continue  # fully masked out: skip tile entirely
            if (t == 1.0).all():
                ent.append((kt, -1, 0, False))
                continue
            nz = np.flatnonzero(t.any(axis=0))
            zq = (int(nz[0]) // P) * P
            # tril-structured tile: [zeros(zq) | tril | ones]
            tri = (
                zq + P <= QC
                and (t[:, :zq] == 0.0).all()
                and (t[:, zq : zq + P] == tril_blk).all()
                and (t[:, zq + P :] == 1.0).all()
            )
            if tri:
                ent.append((kt, -1, zq, True))
                continue
            key = t.tobytes()
            mi = tile_keys.get(key)
            if mi is None:
                mi = len(tiles)
                tile_keys[key] = mi
                tiles.append(t.astype(ml_dtypes.bfloat16))
            ent.append((kt, mi, zq, False))
        for kt, mi, zq, tri in ent:
            for qt in range(zq // P, 4):
                covered[qt] = True
        if ent and not all(covered):
            # some qt block would never be written: disable skipping (the
            # emask multiply zeroes masked probs so pv/dn stay correct)
            ent2 = []
            for kt, mi, zq, tri in ent:
                if zq == 0:
                    ent2.append((kt, mi, zq, tri))
                    continue
                t = emT[P * kt : P * (kt + 1), QC * qc : QC * (qc + 1)]
                key = t.tobytes()
                mi = tile_keys.get(key)
                if mi is None:
                    mi = len(tiles)
                    tile_keys[key] = mi
                    tiles.append(t.astype(ml_dtypes.bfloat16))
                ent2.append((kt, mi, 0, False))
            ent = ent2
        plans.append(tuple(ent))
    if tiles:
        packed = np.ascontiguousarray(np.stack(tiles))
    else:
        packed = np.zeros((1, P, QC), dtype=ml_dtypes.bfloat16)
    return tuple(plans), packed


def kernel(x, mask, section_log_len, wq, wk, wv, wo, seq_scale):
    from concourse.bass_utils import run_bass_kernel_spmd

    x = np.asarray(x, dtype=np.float32)
    assert x.shape == (B, S, D), x.shape
    mask2 = np.asarray(mask, dtype=np.float32).reshape(S, S)
    sll = np.asarray(section_log_len, dtype=np.float32).reshape(S)
    ss = np.asarray(seq_scale, dtype=np.float32).reshape(H)
    wq = np.asarray(wq, dtype=np.float32)
    wk = np.asarray(wk, dtype=np.float32)
    wv = np.asarray(wv, dtype=np.float32)
    wo = np.asarray(wo, dtype=np.float32)

    plans, tiles = _plan_from_mask(mask2)
    key = (plans, tiles.shape[0])
    nc = _GRAPH_CACHE.get(key)
    if nc is None:
        nc = _build_graph(plans, tiles.shape[0])
        _GRAPH_CACHE[key] = nc

    sllB = np.ascontiguousarray(
        np.broadcast_to(sll[None, :], (P, S)), dtype=np.float32
    )
    xT = [_round_f32r(x[b].T) for b in range(B)]
    trilB = np.tril(np.ones((P, P), dtype=np.float32)).astype(ml_dtypes.bfloat16)
    identB = np.eye(P, dtype=np.float32).astype(ml_dtypes.bfloat16)

    in_maps = []
    for c in range(NCORES):
        b, g2 = divmod(c, 4)
        rows = slice(256 * g2, 256 * (g2 + 1))
        ssr = np.repeat(ss[8 * g2 : 8 * g2 + 8], HD) / math.sqrt(HD)
        in_maps.append(
            {
                "xT": xT[b],
                "aq": _round_f32r((wq[rows, :] * ssr[:, None]).T),
                "ak": _round_f32r(wk[rows, :].T),
                "av": _round_f32r(wv[rows, :].T),
                "wor": _round_f32r(wo[:, rows].T),
                "sllb": sllB,
                "tril": trilB,
                "ident": identB,
                "emask": tiles,
            }
        )

    res = run_bass_kernel_spmd(nc, in_maps, core_ids=list(range(NCORES))).results
    out = np.zeros((B, S, D), dtype=np.float32)
    for c in range(NCORES):
        out[c // 4] += np.asarray(res[c]["out"], dtype=np.float32)
    return out


# revision 10
# speedup vs baseline: 1.0476x; 1.0006x over previous
"""Trainium2 Bass kernel for nn_Attention_80917183857290.

Multi-head causal attention (B=2, S=2048, D=1024, H=32, HD=32) with
SSMax-style per-query log-length score scaling, run SPMD on 8 NeuronCores.

Sharding: core c -> batch b = c // 4, head-group g2 = c % 4 (8 heads each).
Per core:
  - projections q,k (transposed layout [head_dim rows, seq]) and v
  - scores^T[k, q] per head via row-tiled K=32 matmuls (4 heads concurrent)
  - probs = exp(scores * sll * ss / sqrt(hd)), masked multiplicatively
  - PV with probs as the stationary operand: out [128q, 32hd] per
    (head, kt) accumulated in PSUM across kt -- the narrow free dim (32)
    makes PV ~4x cheaper on the PE than the scoresT-layout PV.
    Denominators via per-head [128q, 1] matmuls against a ones column.
  - att = pv * recip(dn); att^T via PE transpose; out = att^T.T @ wo_shard.
  - host sums the 4 partial outputs per batch. Dense matmuls use f32r
    (TF32-like single-pass fp32, inputs pre-rounded).

The causal mask reduces to a single shared [128,128] tril tile applied only
on diagonal-crossing blocks; fully-masked [128 k, 128 q] blocks are skipped
in scores/exp/PV entirely. Non-causal masks fall back to per-tile bf16
exp(mask) multiplies (correct for any mask).
"""

import math

import numpy as np
import ml_dtypes

B, S, D, H = 2, 2048, 1024, 32
HD = D // H  # 32
P = 128
QC = 512  # q-chunk (PSUM bank free size, fp32)
NQC = S // QC  # 4
NKT = S // P  # 16
NCORES = 8

_GRAPH_CACHE: dict = {}


def _build_graph(plans, nt, reps=1):
    """Build the per-core Bass graph.

    plans: tuple over qc (4) of tuple of (kt, mi, zq, tri) entries; mi -1
      means no general-mask multiply; >=0 indexes the packed emask tiles;
      tri means multiply the shared [128,128] tril tile on block zq//128.
      zq is a multiple of 128: leading zq q-columns are fully masked.
    nt: number of packed [128, 512] bf16 exp-mask tiles (>= 1).
    """
    import concourse.mybir as mybir
    from concourse import bacc
    from concourse.tile import TileContext

    f32 = mybir.dt.float32
    f32r = mybir.dt.float32r
    bf16 = mybir.dt.bfloat16
    EXP = mybir.ActivationFunctionType.Exp
    MULT = mybir.AluOpType.mult

    nc = bacc.Bacc()

    xT = nc.declare_dram_parameter("xT", [D, S], f32r, isOutput=False)
    aq = nc.declare_dram_parameter("aq", [D, 256], f32r, isOutput=False)
    ak = nc.declare_dram_parameter("ak", [D, 256], f32r, isOutput=False)
    av = nc.declare_dram_parameter("av", [D, 256], f32r, isOutput=False)
    wor = nc.declare_dram_parameter("wor", [256, D], f32r, isOutput=False)
    sllb = nc.declare_dram_parameter("sllb", [P, S], f32, isOutput=False)
    tril = nc.declare_dram_parameter("tril", [P, P], bf16, isOutput=False)
    ident = nc.declare_dram_parameter("ident", [P, P], bf16, isOutput=False)
    emask = nc.declare_dram_parameter("emask", [nt, P, QC], bf16, isOutput=False)
    out = nc.declare_dram_parameter("out", [S, D], bf16, isOutput=True)

    # first/last contributing entry index per (qc, qt) for pv/dn accumulation
    first_e = [[None] * 4 for _ in range(NQC)]
    last_e = [[None] * 4 for _ in range(NQC)]
    for qc in range(NQC):
        for ei, (kt, mi, zq, tri) in enumerate(plans[qc]):
            for qt in range(zq // P, 4):
                if first_e[qc][qt] is None:
                    first_e[qc][qt] = ei
                last_e[qc][qt] = ei

    with TileContext(nc) as tc:
        with (
            tc.tile_pool(name="consts", bufs=1) as consts,
            tc.tile_pool(name="sc", bufs=2, space="PSUM") as sc_pool,
            tc.tile_pool(name="pv", bufs=1, space="PSUM") as pv_pool,
            tc.tile_pool(name="ms", bufs=1, space="PSUM") as ms_pool,
            tc.tile_pool(name="ax", bufs=1, space="PSUM") as ax_pool,
            tc.tile_pool(name="probs", bufs=4) as probs_pool,
            tc.tile_pool(name="emt", bufs=4) as emt_pool,
            tc.tile_pool(name="att", bufs=4) as att_pool,
            tc.tile_pool(name="attT", bufs=2) as attT_pool,
            tc.tile_pool(name="oout", bufs=4) as oout_pool,
            tc.tile_pool(name="recip", bufs=2) as recip_pool,
        ):
          for _rep in range(reps):
            # ---- constant loads (chunked so consumers start early) ----
            aq_sb = consts.tile([P, 8, 256], f32r, tag="aq", name="aq")
            nc.sync.dma_start(out=aq_sb[:], in_=aq.rearrange("(ko ki) f -> ki ko f", ki=P))
            xk = []
            xT_r = xT.rearrange("(ko ki) f -> ki ko f", ki=P)
            for k in range(8):
                t = consts.tile([P, S], f32r, tag=f"xk{k}", name=f"xk{k}")
                nc.sync.dma_start(out=t[:], in_=xT_r[:, k])
                xk.append(t)
            ak_sb = consts.tile([P, 8, 256], f32r, tag="ak", name="ak")
            nc.sync.dma_start(out=ak_sb[:], in_=ak.rearrange("(ko ki) f -> ki ko f", ki=P))
            av_sb = consts.tile([P, 8, 256], f32r, tag="av", name="av")
            nc.sync.dma_start(out=av_sb[:], in_=av.rearrange("(ko ki) f -> ki ko f", ki=P))
            sll_sb = consts.tile([P, S], f32, tag="sll", name="sll")
            nc.sync.dma_start(out=sll_sb[:], in_=sllb[:])
            tril_sb = consts.tile([P, P], bf16, tag="tril", name="tril")
            nc.sync.dma_start(out=tril_sb[:], in_=tril[:])
            id_sb = consts.tile([P, P], bf16, tag="ident", name="ident")
            nc.sync.dma_start(out=id_sb[:], in_=ident[:])
            wor_sb = consts.tile([P, 2, D], f32r, tag="wor", name="wor")
            nc.sync.dma_start(out=wor_sb[:], in_=wor.rearrange("(ko ki) f -> ki ko f", ki=P))
            em_res = None
            if nt <= 16:
                em_res = consts.tile([P, nt, QC], bf16, tag="emres", name="emres")
                nc.sync.dma_start(out=em_res[:], in_=emask.rearrange("t p f -> p t f"))
            ones_sb = consts.tile([P, 1], bf16, tag="ones", name="ones")
            nc.vector.memset(ones_sb[:], 1.0)

            # per-(group, half) q^T/k^T tiles, per-quarter v tiles; chunk
            # granularity lets the scheduler overlap projections with
            # attention on earlier chunks.
            qTh = [[consts.tile([P, 1024], f32r, tag=f"qT{g}{h}", name=f"qT{g}{h}") for h in range(2)] for g in range(2)]
            kTh = [[consts.tile([P, 1024], f32r, tag=f"kT{g}{h}", name=f"kT{g}{h}") for h in range(2)] for g in range(2)]
            vq = [consts.tile([P, 4, 8, 32], bf16, tag=f"vq{q}", name=f"vq{q}") for q in range(4)]

            def proj_qk_unit(w, g, half, c2):
                # one 512-wide chunk of the q or k projection for (g, half)
                lhs_sb = aq_sb if w == "q" else ak_sb
                base = 1024 * half + 512 * c2
                ps = ax_pool.tile([P, 512], f32, tag="ax", name="ax")
                for k in range(8):
                    nc.tensor.matmul(
                        ps[:],
                        lhsT=lhs_sb[:, k, 128 * g : 128 * g + 128],
                        rhs=xk[k][:, base : base + 512],
                        start=(k == 0),
                        stop=(k == 7),
                    )
                dst = (qTh if w == "q" else kTh)[g][half][:, 512 * c2 : 512 * c2 + 512]
                if w == "q":
                    nc.vector.tensor_tensor(dst, ps[:], sll_sb[:, base : base + 512], MULT)
                else:
                    nc.vector.tensor_copy(dst, ps[:])

            def qk_half_units(half):
                return [
                    (lambda w=w, g=g, c2=c2: proj_qk_unit(w, g, half, c2))
                    for g in range(2)
                    for w in ("q", "k")
                    for c2 in range(2)
                ]

            def proj_v_unit(qq, sti):
                st = 4 * qq + sti
                psv = ax_pool.tile([P, 512], f32, tag="ax", name="ax")
                for k in range(8):
                    nc.tensor.matmul(
                        psv[:, :256],
                        lhsT=xk[k][:, 128 * st : 128 * st + 128],
                        rhs=av_sb[:, k, :],
                        start=(k == 0),
                        stop=(k == 7),
                    )
                nc.vector.tensor_copy(
                    vq[qq][:, sti, :, :],
                    psv[:, :256].rearrange("p (h c) -> p h c", h=8),
                )

            def v_quarter_units(qq):
                return [(lambda qq=qq, sti=sti: proj_v_unit(qq, sti)) for sti in range(4)]

            # per-qc state created in attention_qc, consumed by epilogue_*
            pvt = {}   # qc -> (pv01, pv23) tiles [P, 2, 8, 32] f32 PSUM
            mst = {}   # qc -> dn tile [P, 512] f32 PSUM (dn in [:, :32])
            attq = {}  # qc -> list of 4 att tiles [P, 8, 32] bf16 SBUF

            def attention_qc(qc, fillers):
                entries = plans[qc]
                qh, qcol = qc // 2, 512 * (qc % 2)
                pv01 = pv_pool.tile([P, 2, 8, 32], f32, tag="pv01", name="pv01")
                pv23 = pv_pool.tile([P, 2, 8, 32], f32, tag="pv23", name="pv23")
                ms = ms_pool.tile([P, 512], f32, tag="ms", name="ms")
                pvt[qc] = (pv01, pv23)
                mst[qc] = ms
                for g in range(2):
                    for ei, (kt, mi, zq, tri) in enumerate(entries):
                        if fillers and ei % 2 == 1:
                            fillers.popleft()()
                        kh, kcol = kt // 8, 128 * (kt % 8)
                        szq = min(zq, 256)  # keep matmul/exp width >= 256
                        psa = sc_pool.tile([P, 1024], f32, tag="sc", name="sc")
                        psb = sc_pool.tile([P, 1024], f32, tag="sc", name="sc")
                        for j in range(4):
                            dst = (psa if j < 2 else psb)[:, 512 * (j % 2) + szq : 512 * (j % 2) + 512]
                            nc.tensor.matmul(
                                dst,
                                lhsT=kTh[g][kh][32 * j : 32 * j + 32, kcol : kcol + 128],
                                rhs=qTh[g][qh][32 * j : 32 * j + 32, qcol + szq : qcol + 512],
                                start=True,
                                stop=True,
                                tile_position=(32 * j, 0),
                            )
                        pr = probs_pool.tile([P, 2048], bf16, tag="pr", name="pr")
                        if szq == 0:
                            nc.scalar.activation(pr[:, :1024], psa[:], EXP)
                            nc.scalar.activation(pr[:, 1024:], psb[:], EXP)
                        else:
                            for j in range(4):
                                ps_ = (psa if j < 2 else psb)[:, 512 * (j % 2) + szq : 512 * (j % 2) + 512]
                                nc.scalar.activation(pr[:, 512 * j + szq : 512 * j + 512], ps_, EXP)
                        pr4 = pr[:].rearrange("p (h f) -> p h f", h=4)
                        if tri:
                            blk = pr4[:, :, zq : zq + P]
                            nc.vector.tensor_tensor(
                                blk, blk, tril_sb[:, None, :].to_broadcast((P, 4, P)), MULT
                            )
                        elif mi >= 0:
                            if em_res is not None:
                                emt = em_res[:, mi, :]
                            else:
                                emtt = emt_pool.tile([P, QC], bf16, tag="emt", name="emt")
                                nc.sync.dma_start(out=emtt[:], in_=emask[mi])
                                emt = emtt[:]
                            w = QC - zq
                            pr3 = pr4[:, :, zq:]
                            nc.vector.tensor_tensor(
                                pr3, pr3, emt[:, None, zq:].to_broadcast((P, 4, w)), MULT
                            )
                        for j in range(4):
                            h = 4 * g + j
                            for qt in range(zq // P, 4):
                                lhsT = pr[:, 512 * j + 128 * qt : 512 * j + 128 * qt + 128]
                                pvd = (pv01 if qt < 2 else pv23)[:, qt % 2, h, :]
                                nc.tensor.matmul(
                                    pvd,
                                    lhsT=lhsT,
                                    rhs=vq[kt // 4][:, kt % 4, h, :],
                                    start=(ei == first_e[qc][qt]),
                                    stop=(ei == last_e[qc][qt]),
                                    skip_group_check=True,
                                )
                                nc.tensor.matmul(
                                    ms[:, 8 * qt + h : 8 * qt + h + 1],
                                    lhsT=lhsT,
                                    rhs=ones_sb[:],
                                    start=(ei == first_e[qc][qt]),
                                    stop=(ei == last_e[qc][qt]),
                                    skip_group_check=True,
                                )

            def epilogue_divide(qc):
                # recip of the 32 denominators, then per-qt normalize to SBUF
                if not plans[qc]:
                    ats = []
                    for qt in range(4):
                        at = att_pool.tile([P, 8, 32], bf16, tag="at", name="at")
                        nc.vector.memset(at[:], 0.0)
                        ats.append(at)
                    attq[qc] = ats
                    return
                pv01, pv23 = pvt[qc]
                ms = mst[qc]
                rc = recip_pool.tile([P, 32], f32, tag="rc", name="rc")
                nc.vector.reciprocal_approx_fast(out=rc[:], in_=ms[:, :32])
                ats = []
                for qt in range(4):
                    at = att_pool.tile([P, 8, 32], bf16, tag="at", name="at")
                    nc.vector.tensor_tensor(
                        at[:],
                        (pv01 if qt < 2 else pv23)[:, qt % 2],
                        rc[:, 8 * qt : 8 * qt + 8, None].to_broadcast((P, 8, 32)),
                        MULT,
                    )
                    ats.append(at)
                attq[qc] = ats

            def epilogue_wo(qc):
                # transpose att per qt (PE), copy to SBUF f32r, then the
                # output projection; emission staggers transposes one qt
                # ahead of the wo matmuls so the aT copy latency is hidden.
                psTs, aTs = [], []

                def tr(qt):
                    at2 = attq[qc][qt][:].rearrange("p h c -> p (h c)")
                    axt = ax_pool.tile([P, 512], f32, tag="ax", name="ax")
                    psT = axt[:].bitcast(bf16)[:, 0:256].rearrange("p (c q) -> p c q", c=2)
                    for c in range(2):
                        nc.tensor.matmul(
                            psT[:, c],
                            lhsT=at2[:, 128 * c : 128 * c + 128],
                            rhs=id_sb[:],
                            is_transpose=True,
                        )
                    aT = attT_pool.tile([P, 2, 128], f32r, tag="aT", name="aT")
                    nc.vector.tensor_copy(aT[:], psT)
                    aTs.append(aT)

                def wo_qt(qt):
                    aT = aTs[qt]
                    st = 4 * qc + qt
                    for n in range(2):
                        wops = ax_pool.tile([P, 512], f32, tag="ax", name="ax")
                        for kk in range(2):
                            nc.tensor.matmul(
                                wops[:],
                                lhsT=aT[:, kk],
                                rhs=wor_sb[:, kk, 512 * n : 512 * n + 512],
                                start=(kk == 0),
                                stop=(kk == 1),
                            )
                        ob = oout_pool.tile([P, 512], bf16, tag="ob", name="ob")
                        nc.vector.tensor_copy(ob[:], wops[:])
                        nc.sync.dma_start(
                            out=out[128 * st : 128 * st + 128, 512 * n : 512 * n + 512],
                            in_=ob[:],
                        )

                tr(0)
                tr(1)
                wo_qt(0)
                tr(2)
                wo_qt(1)
                tr(3)
                wo_qt(2)
                wo_qt(3)

            # emission order interleaves projections with attention chunks so
            # ACT starts exping as soon as the first chunk's q/k/v exist.
            # Resource-tracked emission: each q/k half and v quarter is
            # emitted as small units, woven between attention kt-iterations
            # of the PREVIOUS chunk where possible so ACT never starves; any
            # units still pending when a chunk actually needs them are
            # drained first (plan-dependent, so non-causal masks stay
            # correct).
            from collections import deque

            done: set = set()
            queued: dict = {}
            fillers: deque = deque()

            def resource_units(r):
                kind, idx = r
                return qk_half_units(idx) if kind == "qk" else v_quarter_units(idx)

            def needs(qc):
                res = [("qk", qc // 2)]
                res += [("qk", kt // 8) for kt, _, _, _ in plans[qc]]
                res += [("v", kt // 4) for kt, _, _, _ in plans[qc]]
                seen = []
                for r in res:
                    if r not in seen:
                        seen.append(r)
                return seen

            def drain(r):
                if r in done:
                    return
                for u in queued.pop(r, None) or resource_units(r):
                    u()
                done.add(r)

            def queue(r):
                if r in done or r in queued:
                    return
                units = resource_units(r)
                queued[r] = []
                fillers.extend(units)
                done.add(r)  # fully queued counts as emitted-by-the-time-we-drain

            prev = None
            for qc in range(NQC):
                for r in needs(qc):
                    # required now: flush any queued-but-unemitted fillers
                    while fillers:
                        fillers.popleft()()
                    drain(r)
                if qc + 1 < NQC:
                    for r in needs(qc + 1):
                        if r not in done:
                            queue(r)
                if prev is not None:
                    epilogue_divide(prev)
                attention_qc(qc, fillers)
                while fillers:
                    fillers.popleft()()
                if prev is not None:
                    epilogue_wo(prev)
                prev = qc
            epilogue_divide(prev)
            epilogue_wo(prev)

    if not nc.is_finalized():
        nc.finalize()
    return nc


def _round_f32r(a):
    """Round fp32 array to the PE's f32r format (mantissa truncated to 11
    bits, round-to-nearest-even at bit 12) so f32r-declared DMA inputs match
    what an on-device cast would produce."""
    u = np.ascontiguousarray(a, dtype=np.float32).view(np.uint32)
    u2 = (u + np.uint32(0x7FF) + ((u >> np.uint32(12)) & np.uint32(1))) & np.uint32(0xFFFFF000)
    return u2.view(np.float32)


def _plan_from_mask(mask):
    """Classify [128, 512] tiles of exp(mask)^T; returns (plans, packed_tiles).

    Entries are (kt, mi, zq, tri): zq (multiple of 128) leading fully-masked
    q-columns; tri=True means the tile is [zeros | tril(128) | ones] so only
    the shared tril block needs multiplying; mi >= 0 indexes a packed general
    bf16 exp(mask) tile.
    """
    em = np.exp(mask.astype(np.float32))  # [q, k]
    emT = np.ascontiguousarray(em.T)  # [k, q]
    # partial diagonal block in [k, q] layout: valid iff q_local >= k_local
    tril_blk = np.triu(np.ones((P, P), dtype=np.float32))
    plans = []
    tiles = []
    tile_keys = {}
    for qc in range(NQC):
        ent = []
        covered = [False] * 4
        for kt in range(NKT):
            t = emT[P * kt : P * (kt + 1), QC * qc : QC * (qc + 1)]
            if not t.any():
                continue  # fully masked out: skip tile entirely
            if (t == 1.0).all():
                ent.append((kt, -1, 0, False))
                continue
            nz = np.flatnonzero(t.any(axis=0))
            zq = (int(nz[0]) // P) * P
            # tril-structured tile: [zeros(zq) | tril | ones]
            tri = (
                zq + P <= QC
                and (t[:, :zq] == 0.0).all()
                and (t[:, zq : zq + P] == tril_blk).all()
                and (t[:, zq + P :] == 1.0).all()
            )
            if tri:
                ent.append((kt, -1, zq, True))
                continue
            key = t.tobytes()
            mi = tile_keys.get(key)
            if mi is None:
                mi = len(tiles)
                tile_keys[key] = mi
                tiles.append(t.astype(ml_dtypes.bfloat16))
            ent.append((kt, mi, zq, False))
        for kt, mi, zq, tri in ent:
            for qt in range(zq // P, 4):
                covered[qt] = True
        if ent and not all(covered):
            # some qt block would never be written: disable skipping (the
            # emask multiply zeroes masked probs so pv/dn stay correct)
            ent2 = []
            for kt, mi, zq, tri in ent:
                if zq == 0:
                    ent2.append((kt, mi, zq, tri))
                    continue
                t = emT[P * kt : P * (kt + 1), QC * qc : QC * (qc + 1)]
                key = t.tobytes()
                mi = tile_keys.get(key)
                if mi is None:
                    mi = len(tiles)
                    tile_keys[key] = mi
                    tiles.append(t.astype(ml_dtypes.bfloat16))
                ent2.append((kt, mi, 0, False))
            ent = ent2
        plans.append(tuple(ent))
    if tiles:
        packed = np.ascontiguousarray(np.stack(tiles))
    else:
        packed = np.zeros((1, P, QC), dtype=ml_dtypes.bfloat16)
    return tuple(plans), packed


def kernel(x, mask, section_log_len, wq, wk, wv, wo, seq_scale):
    from concourse.bass_utils import run_bass_kernel_spmd

    x = np.asarray(x, dtype=np.float32)
    assert x.shape == (B, S, D), x.shape
    mask2 = np.asarray(mask, dtype=np.float32).reshape(S, S)
    sll = np.asarray(section_log_len, dtype=np.float32).reshape(S)
    ss = np.asarray(seq_scale, dtype=np.float32).reshape(H)
    wq = np.asarray(wq, dtype=np.float32)
    wk = np.asarray(wk, dtype=np.float32)
    wv = np.asarray(wv, dtype=np.float32)
    wo = np.asarray(wo, dtype=np.float32)

    plans, tiles = _plan_from_mask(mask2)
    key = (plans, tiles.shape[0])
    nc = _GRAPH_CACHE.get(key)
    if nc is None:
        nc = _build_graph(plans, tiles.shape[0])
        _GRAPH_CACHE[key] = nc

    sllB = np.ascontiguousarray(
        np.broadcast_to(sll[None, :], (P, S)), dtype=np.float32
    )
    xT = [_round_f32r(x[b].T) for b in range(B)]
    trilB = np.triu(np.ones((P, P), dtype=np.float32)).astype(ml_dtypes.bfloat16)
    identB = np.eye(P, dtype=np.float32).astype(ml_dtypes.bfloat16)

    in_maps = []
    for c in range(NCORES):
        b, g2 = divmod(c, 4)
        rows = slice(256 * g2, 256 * (g2 + 1))
        ssr = np.repeat(ss[8 * g2 : 8 * g2 + 8], HD) / math.sqrt(HD)
        in_maps.append(
            {
                "xT": xT[b],
                "aq": _round_f32r((wq[rows, :] * ssr[:, None]).T),
                "ak": _round_f32r(wk[rows, :].T),
                "av": _round_f32r(wv[rows, :].T),
                "wor": _round_f32r(wo[:, rows].T),
                "sllb": sllB,
                "tril": trilB,
                "ident": identB,
                "emask": tiles,
            }
        )

    res = run_bass_kernel_spmd(nc, in_maps, core_ids=list(range(NCORES))).results
    out = np.zeros((B, S, D), dtype=np.float32)
    for c in range(NCORES):
        out[c // 4] += np.asarray(res[c]["out"], dtype=np.float32)
    return out
